# revision 39
# baseline (speedup 1.0000x reference)
"""Trainium2 Bass kernel for a 4-layer dependency GNN (3x GraphConv + GAT).

Full inputs in, full output out. Internally:
  - nodes are sharded across 8 NeuronCores by dst ownership (1250 nodes/core),
  - edges are owned by the core that owns their dst node, sorted by dst and
    chunked into 128-edge groups aligned to 128-node dst tiles,
  - per-layer node features are produced shard-wise, AllGather'ed into a
    per-core DRAM table, and per-edge rows are fetched with dma_gather
    (multi-packet, rotating across 4 SWDGE queues),
  - segment sums run on the tensor engine as one-hot matmuls accumulating
    into PSUM per dst tile; one-hot selection matrices are built in batches
    on the vector engine (is_equal against an iota row),
  - the GAT edge softmax runs unnormalized (exp without max-shift, which is
    mathematically identical) and is normalized per node after aggregation;
    the dst-side attention term er is expanded edge-wise with a transposed
    one-hot matmul instead of a per-edge gather.

Host-side work is limited to index manipulation for the sharding (sorting /
padding / wrapping edge lists, integer degree counts) and data replication of
the small weight tensors; all floating-point model math runs on device.
"""

import sys

import numpy as np

sys.path.insert(0, "/opt/trn_rl_repo")

import concourse.bacc as bacc  # noqa: E402
import concourse.mybir as mybir  # noqa: E402
import concourse.tile as tile  # noqa: E402
from concourse import bass_utils  # noqa: E402
from concourse.masks import make_identity  # noqa: E402

N = 10000
E = 320000
IN_F = 256
HID = 128
HEADS = 4
NCORES = 8
NPC = N // NCORES          # nodes per core
P = 128
NTILES = (NPC + P - 1) // P  # dst node tiles per core
TILE_W = [min(P, NPC - t * P) for t in range(NTILES)]
GB = 7                     # chunk batch: one-hot build / GAT gather granularity
NQ = 4                     # SWDGE queues, rotated across gathers

F32 = mybir.dt.float32
I16 = mybir.dt.int16

# Table dtype for gather tables / one-hots / edge matmuls. bf16 halves gather
# bytes and enables FWL weight loads; accumulation stays fp32 in PSUM.
TABLE_BF16 = True

_compiled_cache = {}


# ----------------------------------------------------------------------------
# host-side sharding / index preprocessing (integer work only)
# ----------------------------------------------------------------------------

def _wrap16(idx_block):
    """dma_gather index layout: [16, n/16] with [p, s] = idx[s*16+p],
    replicated across the 8 gpsimd cores (8 groups of 16 partitions)."""
    n = idx_block.shape[0]
    assert n % 16 == 0
    base = idx_block.reshape(n // 16, 16).T.astype(np.int16)
    return np.tile(base, (8, 1))


def _preprocess(src, dst):
    src = np.asarray(src).astype(np.int64).ravel()
    dst = np.asarray(dst).astype(np.int64).ravel()

    deg_out = np.bincount(src, minlength=N).astype(np.float32)
    deg_in = np.bincount(dst, minlength=N).astype(np.float32)

    per_core = []
    counts = np.zeros((NCORES, NTILES), np.int64)
    groups = {}
    for c in range(NCORES):
        sel = (dst // NPC) == c
        s_c = src[sel]
        d_c = dst[sel] - c * NPC
        order = np.argsort(d_c, kind="stable")
        s_c = s_c[order]
        d_c = d_c[order]
        t_c = d_c // P
        for t in range(NTILES):
            m = t_c == t
            groups[(c, t)] = (s_c[m], d_c[m] - t * P)
            counts[c, t] = m.sum()
    nchunks = max(GB, int(-(-counts.max() // P)))
    nchunks = -(-nchunks // GB) * GB  # multiple of GB keeps batches uniform

    for c in range(NCORES):
        src_blocks, dstloc_blocks = [], []
        for t in range(NTILES):
            s_g, dl_g = groups[(c, t)]
            npad = nchunks * P - len(s_g)
            s_p = np.concatenate([s_g, np.zeros(npad, np.int64)])
            # dst index local to the 128-node tile; 500 = padding sentinel
            dl_p = np.concatenate(
                [dl_g.astype(np.float32), np.full(npad, 500.0, np.float32)]
            )
            src_blocks.append(_wrap16(s_p))
            # one column per chunk: [p, chunk] = dstloc[chunk*128 + p]
            dstloc_blocks.append(dl_p.reshape(nchunks, P).T)
        dstloc = np.concatenate(dstloc_blocks, axis=1).astype(np.float32)
        per_core.append(
            dict(
                src16=np.concatenate(src_blocks, axis=1),
                dstloc=dstloc,
            )
        )

    deg_tiles = []
    for c in range(NCORES):
        do = np.ones((P, NTILES), np.float32)
        di = np.ones((P, NTILES), np.float32)
        for t in range(NTILES):
            w = TILE_W[t]
            base = c * NPC + t * P
            do[:w, t] = deg_out[base : base + w]
            di[:w, t] = deg_in[base : base + w]
        deg_tiles.append((do, di))

    return per_core, deg_tiles, nchunks


# ----------------------------------------------------------------------------
# device program
# ----------------------------------------------------------------------------

def _build(nchunks, stop_after="all"):
    DT = mybir.dt.bfloat16 if TABLE_BF16 else F32
    ZROW = 640 if TABLE_BF16 else 576   # z table row: 512 z + 4 el + pad (256B-mult)
    SROW = 128 if TABLE_BF16 else 64    # s table row: s + pad
    IDXW = nchunks * 8                  # int16 idx cols per tile block
    NB = nchunks // GB                  # chunk batches per tile

    nc = bacc.Bacc(
        "TRN2", target_bir_lowering=False, debug=False, num_devices=NCORES,
        num_swdge_queues=NQ,
    )
    def _splits(n, k):
        step = n // k
        cuts = [i * step for i in range(k)] + [n]
        return [(cuts[i], cuts[i + 1]) for i in range(k) if cuts[i + 1] > cuts[i]]

    _q = [0]

    def next_q():
        _q[0] = (_q[0] + 1) % NQ
        return _q[0]

    # --- I/O ----------------------------------------------------------------
    xT_in = nc.dram_tensor("xT", [IN_F, NPC], F32, kind="ExternalInput")
    w1_in = nc.dram_tensor("w1", [IN_F, HID], F32, kind="ExternalInput")
    w2_in = nc.dram_tensor("w2", [HID, HID], F32, kind="ExternalInput")
    wg_in = nc.dram_tensor("wg", [HID, HEADS * HID], F32, kind="ExternalInput")
    albc_in = nc.dram_tensor("albc", [P, HEADS * HID], F32, kind="ExternalInput")
    arbc_in = nc.dram_tensor("arbc", [P, HEADS * HID], F32, kind="ExternalInput")
    b1bc_in = nc.dram_tensor("b1bc", [P, HID], F32, kind="ExternalInput")
    b2bc_in = nc.dram_tensor("b2bc", [P, HID], F32, kind="ExternalInput")
    bgbc_in = nc.dram_tensor("bgbc", [P, HEADS * HID], F32, kind="ExternalInput")
    w3bc_in = nc.dram_tensor("w3bc", [P, HID], F32, kind="ExternalInput")
    b3bc_in = nc.dram_tensor("b3bc", [P, 1], F32, kind="ExternalInput")
    iota4_in = nc.dram_tensor("iota4", [P, nchunks * P], DT, kind="ExternalInput")
    iotac_in = nc.dram_tensor("iotac", [P, 1], F32, kind="ExternalInput")
    degout_in = nc.dram_tensor("degout", [P, NTILES], F32, kind="ExternalInput")
    degin_in = nc.dram_tensor("degin", [P, NTILES], F32, kind="ExternalInput")
    src16_in = nc.dram_tensor("src16", [P, NTILES * IDXW], I16, kind="ExternalInput")
    dstloc_in = nc.dram_tensor("dstloc", [P, NTILES * nchunks], DT, kind="ExternalInput")
    # per-edge dstloc replicated down partitions (for transposed one-hots)
    dstlocT_in = nc.dram_tensor(
        "dstlocT", [P, NTILES * nchunks * P], DT, kind="ExternalInput"
    )
    risk_out = nc.dram_tensor("risk", [NPC, 1], F32, kind="ExternalOutput")

    rg = [list(range(NCORES))]

    with tile.TileContext(nc) as tc:
        with (
            tc.tile_pool(name="const", bufs=1) as cp,
            tc.tile_pool(name="work", bufs=3) as wp,
            tc.tile_pool(name="gath", bufs=3) as gp,
            tc.tile_pool(name="psum", bufs=2, space="PSUM") as pp,
            tc.tile_pool(name="psum3", bufs=3, space="PSUM") as pp3,
            tc.tile_pool(name="dram", bufs=1, space="DRAM") as dram,
        ):
            # --- DRAM interchange buffers ---------------------------------
            ag_h1 = dram.tile([NPC, HID], DT)
            tab_h1 = dram.tile([N, HID], DT)
            ag_h2 = dram.tile([NPC, HID], DT)
            tab_h2 = dram.tile([N, HID], DT)
            ag_z = dram.tile([NPC, ZROW], DT)
            tab_z = dram.tile([N, ZROW], DT)
            ag_s = dram.tile([NPC, SROW], DT)
            tab_s = dram.tile([N, SROW], DT)

            # --- resident constants ---------------------------------------
            def cload(name, dram_t, shape, dt):
                t = cp.tile(shape, dt, tag=name)
                nc.sync.dma_start(out=t[:], in_=dram_t[:])
                return t

            xT = cp.tile([P, 2, NPC], F32)
            nc.sync.dma_start(out=xT[:], in_=xT_in[:].rearrange("(k p) n -> p k n", p=P))
            w1 = cp.tile([P, 2, HID], F32)
            nc.sync.dma_start(out=w1[:], in_=w1_in[:].rearrange("(k p) f -> p k f", p=P))
            w2 = cload("w2", w2_in, [P, HID], F32)
            wg = cload("wg", wg_in, [P, HEADS * HID], F32)
            albc = cload("albc", albc_in, [P, HEADS * HID], F32)
            arbc = cload("arbc", arbc_in, [P, HEADS * HID], F32)
            b1bc = cload("b1bc", b1bc_in, [P, HID], F32)
            b2bc = cload("b2bc", b2bc_in, [P, HID], F32)
            bgbc = cload("bgbc", bgbc_in, [P, HEADS * HID], F32)
            w3bc = cload("w3bc", w3bc_in, [P, HID], F32)
            b3bc = cload("b3bc", b3bc_in, [P, 1], F32)
            iota4 = cload("iota4", iota4_in, [P, nchunks * P], DT)
            iotac = cload("iotac", iotac_in, [P, 1], F32)
            src16 = cload("src16", src16_in, [P, NTILES * IDXW], I16)
            dstloc = cload("dstloc", dstloc_in, [P, NTILES * nchunks], DT)

            ident = cp.tile([P, P], F32)
            make_identity(nc, ident[:])

            def rsqrt_tile(src_dram):
                d = cp.tile([P, NTILES], F32, tag=f"deg_{src_dram.name}")
                nc.sync.dma_start(out=d[:], in_=src_dram[:])
                nc.vector.tensor_scalar(
                    out=d[:], in0=d[:], scalar1=1.0, scalar2=None,
                    op0=mybir.AluOpType.max,
                )
                nc.vector.reciprocal(out=d[:], in_=d[:])
                nc.scalar.activation(
                    out=d[:], in_=d[:], func=mybir.ActivationFunctionType.Sqrt
                )
                return d

            dsrc = rsqrt_tile(degout_in)
            ddst = rsqrt_tile(degin_in)

            # mean-over-heads of bg: [128, HID]
            bgm = cp.tile([P, HID], F32)
            nc.vector.tensor_tensor(
                out=bgm[:], in0=bgbc[:, 0:HID], in1=bgbc[:, HID : 2 * HID],
                op=mybir.AluOpType.add,
            )
            nc.vector.tensor_tensor(
                out=bgm[:], in0=bgm[:], in1=bgbc[:, 2 * HID : 3 * HID],
                op=mybir.AluOpType.add,
            )
            nc.vector.tensor_tensor(
                out=bgm[:], in0=bgm[:], in1=bgbc[:, 3 * HID : 4 * HID],
                op=mybir.AluOpType.add,
            )
            nc.vector.tensor_scalar(
                out=bgm[:], in0=bgm[:], scalar1=1.0 / HEADS, scalar2=None,
                op0=mybir.AluOpType.mult,
            )

            x2T = cp.tile([P, NTILES * P], F32)
            x3T = cp.tile([P, NTILES * P], F32)
            er_sb = cp.tile([P, NTILES * HEADS], DT)

            # ---------------- helpers ------------------------------------
            def onehot_batch(t, b):
                """[128, GB, 128] DT one-hots for chunks b*GB..(b+1)*GB of tile t."""
                g0 = t * nchunks + b * GB
                oh = wp.tile([P, GB, P], DT, tag="ohb")
                nc.vector.tensor_tensor(
                    out=oh[:],
                    in0=dstloc[:, g0 : g0 + GB].to_broadcast([P, GB, P]),
                    in1=iota4[:, 0 : GB * P].rearrange("p (c j) -> p c j", j=P),
                    op=mybir.AluOpType.is_equal,
                )
                return oh

            _PHASES = [
                "gc1mm", "ag1", "gc1edge", "gc2mm", "ag2", "gc2edge",
                "zphase", "ag3", "gatedge", "ag4", "gc3", "all",
            ]

            def _want(p):
                return _PHASES.index(p) <= _PHASES.index(stop_after)

            def bail():
                z = wp.tile([P, 1], F32, tag="rv")
                nc.vector.memset(z[:], 0.0)
                for t in range(NTILES):
                    w = TILE_W[t]
                    nc.sync.dma_start(
                        out=risk_out[t * P : t * P + w, :], in_=z[:w]
                    )

            def gc_edge_phase(tab, bbc, out_cb):
                """Gather+aggregate a GraphConv layer; out_cb(t, x_tile) consumes
                the [128, HID] f32 post-relu output of dst tile t."""
                for t in range(NTILES):
                    ght = gp.tile([P, nchunks, HID], DT, tag="gc_gather")
                    for c0, c1 in _splits(nchunks, 2):
                        nc.gpsimd.dma_gather(
                            ght[:, c0:c1, :], tab[:],
                            src16[:, t * IDXW + c0 * 8 : t * IDXW + c1 * 8],
                            (c1 - c0) * P, (c1 - c0) * P, HID, elem_step=HID,
                            single_packet=False, queue_num=next_q(),
                        )
                    ps = pp3.tile([P, HID], F32, tag="mm", space="PSUM")
                    for b in range(NB):
                        oh = onehot_batch(t, b)
                        for cc in range(GB):
                            cch = b * GB + cc
                            nc.tensor.matmul(
                                out=ps[:], lhsT=oh[:, cc, :], rhs=ght[:, cch, :],
                                start=(cch == 0), stop=(cch == nchunks - 1),
                            )
                    xt = wp.tile([P, HID], F32, tag="xt")
                    nc.vector.scalar_tensor_tensor(
                        out=xt[:], in0=ps[:], scalar=ddst[:, t : t + 1],
                        in1=bbc[:], op0=mybir.AluOpType.mult,
                        op1=mybir.AluOpType.add,
                    )
                    nc.scalar.activation(
                        out=xt[:], in_=xt[:],
                        func=mybir.ActivationFunctionType.Relu,
                    )
                    out_cb(t, xt)

            def transpose_into(xt, dstT, t):
                pt = pp3.tile([P, P], F32, tag="mm", space="PSUM")
                nc.tensor.transpose(out=pt[:], in_=xt[:], identity=ident[:])
                nc.vector.tensor_copy(out=dstT[:, t * P : (t + 1) * P], in_=pt[:])

            # ---------------- GC1 matmul: h1 = dsrc * (x @ W1) -------------
            for t in range(NTILES):
                w = TILE_W[t]
                ps = pp3.tile([P, HID], F32, tag="mm", space="PSUM")
                for kt in range(2):
                    nc.tensor.matmul(
                        out=ps[:w], lhsT=xT[:, kt, t * P : t * P + w],
                        rhs=w1[:, kt, :], start=(kt == 0), stop=(kt == 1),
                    )
                h1t = wp.tile([P, HID], DT, tag="ht")
                nc.vector.tensor_scalar(
                    out=h1t[:w], in0=ps[:w], scalar1=dsrc[:w, t : t + 1],
                    scalar2=None, op0=mybir.AluOpType.mult,
                )
                nc.sync.dma_start(out=ag_h1[t * P : t * P + w, :], in_=h1t[:w])

            if _want("ag1"):
                nc.gpsimd.collective_compute(
                    "AllGather", mybir.AluOpType.bypass, replica_groups=rg,
                    ins=[ag_h1[:].opt()], outs=[tab_h1[:].opt()],
                )

            # ---------------- GC1 edge + epilogue -> x2 -------------------
            def gc1_out(t, xt):
                transpose_into(xt, x2T, t)

            if _want("gc1edge"):
                gc_edge_phase(tab_h1, b1bc, gc1_out)

            # ---------------- GC2 matmul + AG ------------------------------
            if _want("gc2mm"):
                for t in range(NTILES):
                    w = TILE_W[t]
                    ps = pp3.tile([P, HID], F32, tag="mm", space="PSUM")
                    nc.tensor.matmul(
                        out=ps[:w], lhsT=x2T[:, t * P : t * P + w], rhs=w2[:],
                        start=True, stop=True,
                    )
                    h2t = wp.tile([P, HID], DT, tag="ht")
                    nc.vector.tensor_scalar(
                        out=h2t[:w], in0=ps[:w], scalar1=dsrc[:w, t : t + 1],
                        scalar2=None, op0=mybir.AluOpType.mult,
                    )
                    nc.sync.dma_start(
                        out=ag_h2[t * P : t * P + w, :], in_=h2t[:w]
                    )

            if _want("ag2"):
                nc.gpsimd.collective_compute(
                    "AllGather", mybir.AluOpType.bypass, replica_groups=rg,
                    ins=[ag_h2[:].opt()], outs=[tab_h2[:].opt()],
                )

            # ---------------- GC2 edge -> x3 (transposed) ------------------
            def gc2_out(t, xt):
                transpose_into(xt, x3T, t)

            if _want("gc2edge"):
                gc_edge_phase(tab_h2, b2bc, gc2_out)

            # ---------------- GAT z/el/er + AG -----------------------------
            for t in range(NTILES) if _want("zphase") else []:
                w = TILE_W[t]
                psz = pp3.tile([P, HEADS * HID], F32, tag="mm", space="PSUM")
                nc.tensor.matmul(
                    out=psz[:w], lhsT=x3T[:, t * P : t * P + w], rhs=wg[:],
                    start=True, stop=True,
                )
                zz = wp.tile([P, HEADS * HID], F32, tag="zz")
                el4 = wp.tile([P, HEADS], F32, tag="el4")
                nc.vector.tensor_tensor(
                    out=zz[:], in0=psz[:], in1=albc[:], op=mybir.AluOpType.mult
                )
                nc.vector.reduce_sum(
                    out=el4[:], in_=zz[:].rearrange("p (h d) -> p h d", h=HEADS),
                    axis=mybir.AxisListType.X,
                )
                nc.vector.tensor_tensor(
                    out=zz[:], in0=psz[:], in1=arbc[:], op=mybir.AluOpType.mult
                )
                er4 = wp.tile([P, HEADS], F32, tag="er4")
                nc.vector.reduce_sum(
                    out=er4[:],
                    in_=zz[:].rearrange("p (h d) -> p h d", h=HEADS),
                    axis=mybir.AxisListType.X,
                )
                nc.vector.tensor_copy(
                    out=er_sb[:, t * HEADS : (t + 1) * HEADS], in_=er4[:]
                )
                zst = wp.tile([P, ZROW], DT, tag="zst")
                nc.vector.tensor_copy(out=zst[:, 0 : HEADS * HID], in_=psz[:])
                nc.vector.tensor_copy(
                    out=zst[:, HEADS * HID : HEADS * HID + HEADS], in_=el4[:]
                )
                nc.sync.dma_start(out=ag_z[t * P : t * P + w, :], in_=zst[:w])

            if _want("ag3"):
                nc.gpsimd.collective_compute(
                    "AllGather", mybir.AluOpType.bypass, replica_groups=rg,
                    ins=[ag_z[:].opt()], outs=[tab_z[:].opt()],
                )

            # ---------------- GAT edge phase ------------------------------
            for t in range(NTILES) if _want("gatedge") else []:
                w = TILE_W[t]
                ps_out = pp3.tile([P, HEADS * HID], F32, tag="mm", space="PSUM")
                ps_den = pp3.tile([P, HEADS], F32, tag="den", space="PSUM")
                ert = er_sb[:, t * HEADS : (t + 1) * HEADS]
                for b in range(NB):
                    i0 = t * IDXW + b * GB * 8
                    zg = gp.tile([P, GB, ZROW], DT, tag="zg")
                    nc.gpsimd.dma_gather(
                        zg[:], tab_z[:], src16[:, i0 : i0 + GB * 8],
                        GB * P, GB * P, ZROW, elem_step=ZROW,
                        single_packet=False, queue_num=next_q(),
                    )
                    # transposed one-hots for the er expansion
                    g0 = (t * nchunks + b * GB) * P
                    dlT = gp.tile([P, GB * P], DT, tag="dlT")
                    nc.sync.dma_start(
                        out=dlT[:], in_=dstlocT_in[:, g0 : g0 + GB * P]
                    )
                    ohT = wp.tile([P, GB * P], DT, tag="ohT")
                    nc.vector.tensor_scalar(
                        out=ohT[:], in0=dlT[:], scalar1=iotac[:, 0:1],
                        scalar2=None, op0=mybir.AluOpType.is_equal,
                    )
                    ps_er = pp.tile([P, GB * HEADS], F32, tag="er", space="PSUM")
                    for cc in range(GB):
                        nc.tensor.matmul(
                            out=ps_er[:, cc * HEADS : (cc + 1) * HEADS],
                            lhsT=ohT[:, cc * P : (cc + 1) * P], rhs=ert,
                            start=True, stop=True, skip_group_check=True,
                        )
                    # batched ex = exp(lrelu(el + er))  [128, GB*4]
                    e_all = wp.tile([P, GB * HEADS], F32, tag="e_all")
                    ex_all = wp.tile([P, GB * HEADS], DT, tag="ex_all")
                    nc.vector.tensor_tensor(
                        out=e_all[:].rearrange("p (c h) -> p c h", h=HEADS),
                        in0=zg[:, :, HEADS * HID : HEADS * HID + HEADS],
                        in1=ps_er[:].rearrange("p (c h) -> p c h", h=HEADS),
                        op=mybir.AluOpType.add,
                    )
                    nc.vector.scalar_tensor_tensor(
                        out=e_all[:], in0=e_all[:], scalar=0.2,
                        in1=e_all[:], op0=mybir.AluOpType.mult,
                        op1=mybir.AluOpType.max,
                    )
                    nc.scalar.activation(
                        out=ex_all[:], in_=e_all[:],
                        func=mybir.ActivationFunctionType.Exp,
                    )
                    oh = onehot_batch(t, b)
                    for cc in range(GB):
                        cch = b * GB + cc
                        first = cch == 0
                        last = cch == nchunks - 1
                        zw = wp.tile([P, HEADS * HID], DT, tag="zw")
                        nc.vector.tensor_tensor(
                            out=zw[:].rearrange("p (h d) -> p h d", h=HEADS),
                            in0=zg[:, cc, 0 : HEADS * HID].rearrange(
                                "p (h d) -> p h d", h=HEADS
                            ),
                            in1=ex_all[:, cc * HEADS : (cc + 1) * HEADS]
                            .to_broadcast([P, HEADS, HID]),
                            op=mybir.AluOpType.mult,
                        )
                        nc.tensor.matmul(
                            out=ps_out[:], lhsT=oh[:, cc, :], rhs=zw[:],
                            start=first, stop=last,
                        )
                        nc.tensor.matmul(
                            out=ps_den[:], lhsT=oh[:, cc, :],
                            rhs=ex_all[:, cc * HEADS : (cc + 1) * HEADS],
                            start=first, stop=last,
                        )
                # epilogue: out = ps_out / den; x4 = mean_h + bg_mean
                den = wp.tile([P, HEADS], F32, tag="den_sb")
                nc.vector.tensor_scalar(
                    out=den[:], in0=ps_den[:], scalar1=1e-30, scalar2=None,
                    op0=mybir.AluOpType.max,
                )
                rden = wp.tile([P, HEADS], F32, tag="rden")
                nc.vector.reciprocal(out=rden[:], in_=den[:])
                outw = wp.tile([P, HEADS * HID], F32, tag="outw")
                nc.vector.tensor_tensor(
                    out=outw[:].rearrange("p (h d) -> p h d", h=HEADS),
                    in0=ps_out[:].rearrange("p (h d) -> p h d", h=HEADS),
                    in1=rden[:].to_broadcast([P, HEADS, HID]),
                    op=mybir.AluOpType.mult,
                )
                hsum = wp.tile([P, HID], F32, tag="hsum")
                nc.vector.tensor_tensor(
                    out=hsum[:], in0=outw[:, 0:HID], in1=outw[:, HID : 2 * HID],
                    op=mybir.AluOpType.add,
                )
                nc.vector.tensor_tensor(
                    out=hsum[:], in0=hsum[:], in1=outw[:, 2 * HID : 3 * HID],
                    op=mybir.AluOpType.add,
                )
                nc.vector.tensor_tensor(
                    out=hsum[:], in0=hsum[:], in1=outw[:, 3 * HID : 4 * HID],
                    op=mybir.AluOpType.add,
                )
                x4 = wp.tile([P, HID], F32, tag="x4")
                nc.vector.scalar_tensor_tensor(
                    out=x4[:], in0=hsum[:], scalar=1.0 / HEADS, in1=bgm[:],
                    op0=mybir.AluOpType.mult, op1=mybir.AluOpType.add,
                )
                nc.vector.tensor_scalar(
                    out=x4[:], in0=x4[:], scalar1=dsrc[:, t : t + 1],
                    scalar2=None, op0=mybir.AluOpType.mult,
                )
                m3 = wp.tile([P, HID], F32, tag="m3")
                nc.vector.tensor_tensor(
                    out=m3[:], in0=x4[:], in1=w3bc[:], op=mybir.AluOpType.mult
                )
                sv = wp.tile([P, 1], F32, tag="sv")
                nc.vector.reduce_sum(out=sv[:], in_=m3[:], axis=mybir.AxisListType.X)
                sst = wp.tile([P, SROW], DT, tag="sst")
                nc.vector.tensor_copy(out=sst[:, 0:1], in_=sv[:])
                nc.sync.dma_start(out=ag_s[t * P : t * P + w, :], in_=sst[:w])

            if _want("ag4"):
                nc.gpsimd.collective_compute(
                    "AllGather", mybir.AluOpType.bypass, replica_groups=rg,
                    ins=[ag_s[:].opt()], outs=[tab_s[:].opt()],
                )

            # ---------------- GC3 edge + sigmoid --------------------------
            for t in range(NTILES) if _want("gc3") else []:
                w = TILE_W[t]
                sg = gp.tile([P, nchunks, SROW], DT, tag="sg")
                for c0, c1 in _splits(nchunks, 2):
                    nc.gpsimd.dma_gather(
                        sg[:, c0:c1, :], tab_s[:],
                        src16[:, t * IDXW + c0 * 8 : t * IDXW + c1 * 8],
                        (c1 - c0) * P, (c1 - c0) * P, SROW, elem_step=SROW,
                        single_packet=False, queue_num=next_q(),
                    )
                ps = pp3.tile([P, 1], F32, tag="den", space="PSUM")
                for b in range(NB):
                    oh = onehot_batch(t, b)
                    for cc in range(GB):
                        cch = b * GB + cc
                        nc.tensor.matmul(
                            out=ps[:], lhsT=oh[:, cc, :], rhs=sg[:, cch, 0:1],
                            start=(cch == 0), stop=(cch == nchunks - 1),
                        )
                rv = wp.tile([P, 1], F32, tag="rv")
                nc.vector.tensor_scalar(
                    out=rv[:], in0=ps[:], scalar1=ddst[:, t : t + 1],
                    scalar2=None, op0=mybir.AluOpType.mult,
                )
                nc.scalar.activation(
                    out=rv[:], in_=rv[:],
                    func=mybir.ActivationFunctionType.Sigmoid,
                    bias=b3bc[:, 0:1], scale=1.0,
                )
                nc.sync.dma_start(out=risk_out[t * P : t * P + w, :], in_=rv[:w])

            if stop_after != "all":
                bail()

    nc.compile()
    return nc


# ----------------------------------------------------------------------------
# host driver
# ----------------------------------------------------------------------------

def _get_program(nchunks):
    if nchunks not in _compiled_cache:
        _compiled_cache[nchunks] = _build(nchunks)
    return _compiled_cache[nchunks]


def _install_ntff_hook():
    """Profiling support: register the NTFF hook bass_utils expects when this
    image's antenv package lacks axon_hooks. Best-effort, trace-path only."""
    import types

    try:
        import antenv.axon_hooks  # noqa: F401

        return
    except ImportError:
        pass
    try:
        import antenv
        from trn_agent_boot.trn_boot import _ntff_profile_via_ctypes

        hook = _ntff_profile_via_ctypes("/opt/axon/libaxon_pjrt.so")
        mod = types.ModuleType("antenv.axon_hooks")
        mod.get_axon_ntff_profile_hook = lambda: hook
        mod.set_axon_ntff_profile_hook = lambda h: None
        sys.modules["antenv.axon_hooks"] = mod
        antenv.axon_hooks = mod
    except Exception:
        pass


def _to_table_dtype(a):
    if TABLE_BF16:
        import ml_dtypes

        return a.astype(ml_dtypes.bfloat16)
    return a.astype(np.float32)


def kernel(
    features, src, dst, W1, b1, W2, b2, W3, b3, Wg, attn_l, attn_r, bg,
    _trace=False,
):
    features = np.asarray(features, np.float32)
    per_core, deg_tiles, nchunks = _preprocess(src, dst)
    nc = _get_program(nchunks)

    iota4 = np.tile(np.arange(P, dtype=np.float32), nchunks)[None, :].repeat(P, 0)
    iotac = np.arange(P, dtype=np.float32)[:, None]
    common = dict(
        w1=np.asarray(W1, np.float32),
        w2=np.asarray(W2, np.float32),
        wg=np.asarray(Wg, np.float32),
        albc=np.tile(np.asarray(attn_l, np.float32).reshape(1, -1), (P, 1)),
        arbc=np.tile(np.asarray(attn_r, np.float32).reshape(1, -1), (P, 1)),
        b1bc=np.tile(np.asarray(b1, np.float32).reshape(1, -1), (P, 1)),
        b2bc=np.tile(np.asarray(b2, np.float32).reshape(1, -1), (P, 1)),
        bgbc=np.tile(np.asarray(bg, np.float32).reshape(1, -1), (P, 1)),
        w3bc=np.tile(np.asarray(W3, np.float32).reshape(1, -1), (P, 1)),
        b3bc=np.full((P, 1), np.float32(np.asarray(b3).reshape(-1)[0])),
        iota4=_to_table_dtype(iota4),
        iotac=iotac.astype(np.float32),
    )
    in_maps = []
    for c in range(NCORES):
        m = dict(common)
        m["xT"] = np.ascontiguousarray(features[c * NPC : (c + 1) * NPC].T)
        m["degout"], m["degin"] = deg_tiles[c]
        m["src16"] = per_core[c]["src16"]
        m["dstloc"] = _to_table_dtype(per_core[c]["dstloc"])
        # per-edge dstloc in wrapped-by-chunk row layout, replicated down parts
        dl = per_core[c]["dstloc"]  # [128, NTILES*nchunks], [p, g] = edge g*128+p
        row = dl.T.reshape(1, -1)   # [1, NTILES*nchunks*128] edge-major
        m["dstlocT"] = _to_table_dtype(np.repeat(row, P, axis=0))
        in_maps.append(m)

    if _trace:
        _install_ntff_hook()
    res = bass_utils.run_bass_kernel_spmd(
        nc, in_maps, core_ids=list(range(NCORES)), trace=_trace
    )
    out = np.concatenate([res.results[c]["risk"] for c in range(NCORES)], axis=0)
    if _trace:
        kernel.last_exec_time_ns = res.exec_time_ns
        kernel.last_results = res
    return out.astype(np.float32)


# revision 40
# speedup vs baseline: 1.0176x; 1.0176x over previous
"""Trainium2 Bass kernel for a 4-layer dependency GNN (3x GraphConv + GAT).

Full inputs in, full output out. Internally:
  - nodes are sharded across 8 NeuronCores by dst ownership (1250 nodes/core),
  - edges are owned by the core that owns their dst node, sorted by dst and
    chunked into 128-edge groups aligned to 128-node dst tiles,
  - per-layer node features are produced shard-wise, AllGather'ed into a
    per-core DRAM table, and per-edge rows are fetched with dma_gather
    (multi-packet, rotating across 4 SWDGE queues),
  - segment sums run on the tensor engine as one-hot matmuls accumulating
    into PSUM per dst tile; one-hot selection matrices are built in batches
    on the vector engine (is_equal against an iota row),
  - the GAT edge softmax runs unnormalized (exp without max-shift, which is
    mathematically identical) and is normalized per node after aggregation;
    the dst-side attention term er is expanded edge-wise with a transposed
    one-hot matmul instead of a per-edge gather.

Host-side work is limited to index manipulation for the sharding (sorting /
padding / wrapping edge lists, integer degree counts) and data replication of
the small weight tensors; all floating-point model math runs on device.
"""

import sys

import numpy as np

sys.path.insert(0, "/opt/trn_rl_repo")

import concourse.bacc as bacc  # noqa: E402
import concourse.mybir as mybir  # noqa: E402
import concourse.tile as tile  # noqa: E402
from concourse import bass_utils  # noqa: E402
from concourse.masks import make_identity  # noqa: E402

N = 10000
E = 320000
IN_F = 256
HID = 128
HEADS = 4
NCORES = 8
NPC = N // NCORES          # nodes per core
P = 128
NTILES = (NPC + P - 1) // P  # dst node tiles per core
TILE_W = [min(P, NPC - t * P) for t in range(NTILES)]
GB = 7                     # chunk batch: one-hot build / GAT gather granularity
NQ = 4                     # SWDGE queues, rotated across gathers

F32 = mybir.dt.float32
I16 = mybir.dt.int16

# Table dtype for gather tables / one-hots / edge matmuls. bf16 halves gather
# bytes and enables FWL weight loads; accumulation stays fp32 in PSUM.
TABLE_BF16 = True

_compiled_cache = {}


# ----------------------------------------------------------------------------
# host-side sharding / index preprocessing (integer work only)
# ----------------------------------------------------------------------------

def _wrap16(idx_block):
    """dma_gather index layout: [16, n/16] with [p, s] = idx[s*16+p],
    replicated across the 8 gpsimd cores (8 groups of 16 partitions)."""
    n = idx_block.shape[0]
    assert n % 16 == 0
    base = idx_block.reshape(n // 16, 16).T.astype(np.int16)
    return np.tile(base, (8, 1))


def _preprocess(src, dst):
    src = np.asarray(src).astype(np.int64).ravel()
    dst = np.asarray(dst).astype(np.int64).ravel()

    deg_out = np.bincount(src, minlength=N).astype(np.float32)
    deg_in = np.bincount(dst, minlength=N).astype(np.float32)

    per_core = []
    counts = np.zeros((NCORES, NTILES), np.int64)
    groups = {}
    for c in range(NCORES):
        sel = (dst // NPC) == c
        s_c = src[sel]
        d_c = dst[sel] - c * NPC
        order = np.argsort(d_c, kind="stable")
        s_c = s_c[order]
        d_c = d_c[order]
        t_c = d_c // P
        for t in range(NTILES):
            m = t_c == t
            groups[(c, t)] = (s_c[m], d_c[m] - t * P)
            counts[c, t] = m.sum()
    nchunks = max(GB, int(-(-counts.max() // P)))
    nchunks = -(-nchunks // GB) * GB  # multiple of GB keeps batches uniform

    for c in range(NCORES):
        src_blocks, dstloc_blocks = [], []
        for t in range(NTILES):
            s_g, dl_g = groups[(c, t)]
            npad = nchunks * P - len(s_g)
            s_p = np.concatenate([s_g, np.zeros(npad, np.int64)])
            # dst index local to the 128-node tile; 500 = padding sentinel
            dl_p = np.concatenate(
                [dl_g.astype(np.float32), np.full(npad, 500.0, np.float32)]
            )
            src_blocks.append(_wrap16(s_p))
            # one column per chunk: [p, chunk] = dstloc[chunk*128 + p]
            dstloc_blocks.append(dl_p.reshape(nchunks, P).T)
        dstloc = np.concatenate(dstloc_blocks, axis=1).astype(np.float32)
        per_core.append(
            dict(
                src16=np.concatenate(src_blocks, axis=1),
                dstloc=dstloc,
            )
        )

    deg_tiles = []
    for c in range(NCORES):
        do = np.ones((P, NTILES), np.float32)
        di = np.ones((P, NTILES), np.float32)
        for t in range(NTILES):
            w = TILE_W[t]
            base = c * NPC + t * P
            do[:w, t] = deg_out[base : base + w]
            di[:w, t] = deg_in[base : base + w]
        deg_tiles.append((do, di))

    return per_core, deg_tiles, nchunks


# ----------------------------------------------------------------------------
# device program
# ----------------------------------------------------------------------------

def _build(nchunks, stop_after="all"):
    DT = mybir.dt.bfloat16 if TABLE_BF16 else F32
    ZROW = 640 if TABLE_BF16 else 576   # z table row: 512 z + 4 el + pad (256B-mult)
    SROW = 128 if TABLE_BF16 else 64    # s table row: s + pad
    IDXW = nchunks * 8                  # int16 idx cols per tile block
    NB = nchunks // GB                  # chunk batches per tile

    nc = bacc.Bacc(
        "TRN2", target_bir_lowering=False, debug=False, num_devices=NCORES,
        num_swdge_queues=NQ,
    )
    def _splits(n, k):
        step = n // k
        cuts = [i * step for i in range(k)] + [n]
        return [(cuts[i], cuts[i + 1]) for i in range(k) if cuts[i + 1] > cuts[i]]

    _q = [0]

    def next_q():
        _q[0] = (_q[0] + 1) % NQ
        return _q[0]

    # --- I/O ----------------------------------------------------------------
    xT_in = nc.dram_tensor("xT", [IN_F, NPC], F32, kind="ExternalInput")
    w1_in = nc.dram_tensor("w1", [IN_F, HID], F32, kind="ExternalInput")
    w2_in = nc.dram_tensor("w2", [HID, HID], F32, kind="ExternalInput")
    wg_in = nc.dram_tensor("wg", [HID, HEADS * HID], F32, kind="ExternalInput")
    albc_in = nc.dram_tensor("albc", [P, HEADS * HID], F32, kind="ExternalInput")
    arbc_in = nc.dram_tensor("arbc", [P, HEADS * HID], F32, kind="ExternalInput")
    b1bc_in = nc.dram_tensor("b1bc", [P, HID], F32, kind="ExternalInput")
    b2bc_in = nc.dram_tensor("b2bc", [P, HID], F32, kind="ExternalInput")
    bgbc_in = nc.dram_tensor("bgbc", [P, HEADS * HID], F32, kind="ExternalInput")
    w3bc_in = nc.dram_tensor("w3bc", [P, HID], F32, kind="ExternalInput")
    b3bc_in = nc.dram_tensor("b3bc", [P, 1], F32, kind="ExternalInput")
    iota4_in = nc.dram_tensor("iota4", [P, nchunks * P], DT, kind="ExternalInput")
    iotac_in = nc.dram_tensor("iotac", [P, 1], F32, kind="ExternalInput")
    degout_in = nc.dram_tensor("degout", [P, NTILES], F32, kind="ExternalInput")
    degin_in = nc.dram_tensor("degin", [P, NTILES], F32, kind="ExternalInput")
    src16_in = nc.dram_tensor("src16", [P, NTILES * IDXW], I16, kind="ExternalInput")
    dstloc_in = nc.dram_tensor("dstloc", [P, NTILES * nchunks], DT, kind="ExternalInput")
    # per-edge dstloc replicated down partitions (for transposed one-hots)
    dstlocT_in = nc.dram_tensor(
        "dstlocT", [P, NTILES * nchunks * P], DT, kind="ExternalInput"
    )
    risk_out = nc.dram_tensor("risk", [NPC, 1], F32, kind="ExternalOutput")

    rg = [list(range(NCORES))]

    with tile.TileContext(nc) as tc:
        with (
            tc.tile_pool(name="const", bufs=1) as cp,
            tc.tile_pool(name="work", bufs=3) as wp,
            tc.tile_pool(name="gath", bufs=3) as gp,
            tc.tile_pool(name="psum", bufs=2, space="PSUM") as pp,
            tc.tile_pool(name="psum3", bufs=3, space="PSUM") as pp3,
            tc.tile_pool(name="dram", bufs=1, space="DRAM") as dram,
        ):
            # --- DRAM interchange buffers ---------------------------------
            ag_h1 = dram.tile([NPC, HID], DT)
            tab_h1 = dram.tile([N, HID], DT)
            ag_h2 = dram.tile([NPC, HID], DT)
            tab_h2 = dram.tile([N, HID], DT)
            ag_z = dram.tile([NPC, ZROW], DT)
            tab_z = dram.tile([N, ZROW], DT)
            ag_s = dram.tile([NPC, SROW], DT)
            tab_s = dram.tile([N, SROW], DT)

            # --- resident constants ---------------------------------------
            def cload(name, dram_t, shape, dt):
                t = cp.tile(shape, dt, tag=name)
                nc.sync.dma_start(out=t[:], in_=dram_t[:])
                return t

            xT = cp.tile([P, 2, NPC], F32)
            nc.sync.dma_start(out=xT[:], in_=xT_in[:].rearrange("(k p) n -> p k n", p=P))
            w1 = cp.tile([P, 2, HID], F32)
            nc.sync.dma_start(out=w1[:], in_=w1_in[:].rearrange("(k p) f -> p k f", p=P))
            w2 = cload("w2", w2_in, [P, HID], F32)
            wg = cload("wg", wg_in, [P, HEADS * HID], F32)
            albc = cload("albc", albc_in, [P, HEADS * HID], F32)
            arbc = cload("arbc", arbc_in, [P, HEADS * HID], F32)
            b1bc = cload("b1bc", b1bc_in, [P, HID], F32)
            b2bc = cload("b2bc", b2bc_in, [P, HID], F32)
            bgbc = cload("bgbc", bgbc_in, [P, HEADS * HID], F32)
            w3bc = cload("w3bc", w3bc_in, [P, HID], F32)
            b3bc = cload("b3bc", b3bc_in, [P, 1], F32)
            iota4 = cload("iota4", iota4_in, [P, nchunks * P], DT)
            iotac = cload("iotac", iotac_in, [P, 1], F32)
            src16 = cload("src16", src16_in, [P, NTILES * IDXW], I16)
            dstloc = cload("dstloc", dstloc_in, [P, NTILES * nchunks], DT)

            ident = cp.tile([P, P], F32)
            make_identity(nc, ident[:])

            def rsqrt_tile(src_dram):
                d = cp.tile([P, NTILES], F32, tag=f"deg_{src_dram.name}")
                nc.sync.dma_start(out=d[:], in_=src_dram[:])
                nc.vector.tensor_scalar(
                    out=d[:], in0=d[:], scalar1=1.0, scalar2=None,
                    op0=mybir.AluOpType.max,
                )
                nc.vector.reciprocal(out=d[:], in_=d[:])
                nc.scalar.activation(
                    out=d[:], in_=d[:], func=mybir.ActivationFunctionType.Sqrt
                )
                return d

            dsrc = rsqrt_tile(degout_in)
            ddst = rsqrt_tile(degin_in)

            # mean-over-heads of bg: [128, HID]
            bgm = cp.tile([P, HID], F32)
            nc.vector.tensor_tensor(
                out=bgm[:], in0=bgbc[:, 0:HID], in1=bgbc[:, HID : 2 * HID],
                op=mybir.AluOpType.add,
            )
            nc.vector.tensor_tensor(
                out=bgm[:], in0=bgm[:], in1=bgbc[:, 2 * HID : 3 * HID],
                op=mybir.AluOpType.add,
            )
            nc.vector.tensor_tensor(
                out=bgm[:], in0=bgm[:], in1=bgbc[:, 3 * HID : 4 * HID],
                op=mybir.AluOpType.add,
            )
            nc.vector.tensor_scalar(
                out=bgm[:], in0=bgm[:], scalar1=1.0 / HEADS, scalar2=None,
                op0=mybir.AluOpType.mult,
            )

            x2T = cp.tile([P, NTILES * P], F32)
            x3T = cp.tile([P, NTILES * P], F32)
            er_sb = cp.tile([P, NTILES * HEADS], DT)

            # ---------------- helpers ------------------------------------
            def onehot_batch(t, b):
                """[128, GB, 128] DT one-hots for chunks b*GB..(b+1)*GB of tile t."""
                g0 = t * nchunks + b * GB
                oh = wp.tile([P, GB, P], DT, tag="ohb")
                nc.vector.tensor_tensor(
                    out=oh[:],
                    in0=dstloc[:, g0 : g0 + GB].to_broadcast([P, GB, P]),
                    in1=iota4[:, 0 : GB * P].rearrange("p (c j) -> p c j", j=P),
                    op=mybir.AluOpType.is_equal,
                )
                return oh

            _PHASES = [
                "gc1mm", "ag1", "gc1edge", "gc2mm", "ag2", "gc2edge",
                "zphase", "ag3", "gatedge", "ag4", "gc3", "all",
            ]

            def _want(p):
                return _PHASES.index(p) <= _PHASES.index(stop_after)

            def bail():
                z = wp.tile([P, 1], F32, tag="rv")
                nc.vector.memset(z[:], 0.0)
                for t in range(NTILES):
                    w = TILE_W[t]
                    nc.sync.dma_start(
                        out=risk_out[t * P : t * P + w, :], in_=z[:w]
                    )

            def gc_edge_phase(tab, bbc, out_cb):
                """Gather+aggregate a GraphConv layer; out_cb(t, x_tile) consumes
                the [128, HID] f32 post-relu output of dst tile t."""
                for t in range(NTILES):
                    ght = gp.tile([P, nchunks, HID], DT, tag="gc_gather")
                    for c0, c1 in _splits(nchunks, 2):
                        nc.gpsimd.dma_gather(
                            ght[:, c0:c1, :], tab[:],
                            src16[:, t * IDXW + c0 * 8 : t * IDXW + c1 * 8],
                            (c1 - c0) * P, (c1 - c0) * P, HID, elem_step=HID,
                            single_packet=False, queue_num=next_q(),
                        )
                    ps = pp3.tile([P, HID], F32, tag="mm", space="PSUM")
                    for b in range(NB):
                        oh = onehot_batch(t, b)
                        for cc in range(GB):
                            cch = b * GB + cc
                            nc.tensor.matmul(
                                out=ps[:], lhsT=oh[:, cc, :], rhs=ght[:, cch, :],
                                start=(cch == 0), stop=(cch == nchunks - 1),
                            )
                    xt = wp.tile([P, HID], F32, tag="xt")
                    nc.vector.scalar_tensor_tensor(
                        out=xt[:], in0=ps[:], scalar=ddst[:, t : t + 1],
                        in1=bbc[:], op0=mybir.AluOpType.mult,
                        op1=mybir.AluOpType.add,
                    )
                    nc.scalar.activation(
                        out=xt[:], in_=xt[:],
                        func=mybir.ActivationFunctionType.Relu,
                    )
                    out_cb(t, xt)

            def transpose_into(xt, dstT, t):
                pt = pp3.tile([P, P], F32, tag="mm", space="PSUM")
                nc.tensor.transpose(out=pt[:], in_=xt[:], identity=ident[:])
                nc.vector.tensor_copy(out=dstT[:, t * P : (t + 1) * P], in_=pt[:])

            # ---------------- GC1 matmul: h1 = dsrc * (x @ W1) -------------
            for t in range(NTILES):
                w = TILE_W[t]
                ps = pp3.tile([P, HID], F32, tag="mm", space="PSUM")
                for kt in range(2):
                    nc.tensor.matmul(
                        out=ps[:w], lhsT=xT[:, kt, t * P : t * P + w],
                        rhs=w1[:, kt, :], start=(kt == 0), stop=(kt == 1),
                    )
                h1t = wp.tile([P, HID], DT, tag="ht")
                nc.vector.tensor_scalar(
                    out=h1t[:w], in0=ps[:w], scalar1=dsrc[:w, t : t + 1],
                    scalar2=None, op0=mybir.AluOpType.mult,
                )
                nc.sync.dma_start(out=ag_h1[t * P : t * P + w, :], in_=h1t[:w])

            if _want("ag1"):
                nc.gpsimd.collective_compute(
                    "AllGather", mybir.AluOpType.bypass, replica_groups=rg,
                    ins=[ag_h1[:].opt()], outs=[tab_h1[:].opt()],
                )

            # ---------------- GC1 edge + epilogue -> x2 -------------------
            def gc1_out(t, xt):
                transpose_into(xt, x2T, t)

            if _want("gc1edge"):
                gc_edge_phase(tab_h1, b1bc, gc1_out)

            # ---------------- GC2 matmul + AG ------------------------------
            if _want("gc2mm"):
                for t in range(NTILES):
                    w = TILE_W[t]
                    ps = pp3.tile([P, HID], F32, tag="mm", space="PSUM")
                    nc.tensor.matmul(
                        out=ps[:w], lhsT=x2T[:, t * P : t * P + w], rhs=w2[:],
                        start=True, stop=True,
                    )
                    h2t = wp.tile([P, HID], DT, tag="ht")
                    nc.vector.tensor_scalar(
                        out=h2t[:w], in0=ps[:w], scalar1=dsrc[:w, t : t + 1],
                        scalar2=None, op0=mybir.AluOpType.mult,
                    )
                    nc.sync.dma_start(
                        out=ag_h2[t * P : t * P + w, :], in_=h2t[:w]
                    )

            if _want("ag2"):
                nc.gpsimd.collective_compute(
                    "AllGather", mybir.AluOpType.bypass, replica_groups=rg,
                    ins=[ag_h2[:].opt()], outs=[tab_h2[:].opt()],
                )

            # ---------------- GC2 edge -> x3 (transposed) ------------------
            def gc2_out(t, xt):
                transpose_into(xt, x3T, t)

            if _want("gc2edge"):
                gc_edge_phase(tab_h2, b2bc, gc2_out)

            # ---------------- GAT z/el/er + AG -----------------------------
            for t in range(NTILES) if _want("zphase") else []:
                w = TILE_W[t]
                psz = pp3.tile([P, HEADS * HID], F32, tag="mm", space="PSUM")
                nc.tensor.matmul(
                    out=psz[:w], lhsT=x3T[:, t * P : t * P + w], rhs=wg[:],
                    start=True, stop=True,
                )
                zz = wp.tile([P, HEADS * HID], F32, tag="zz")
                el4 = wp.tile([P, HEADS], F32, tag="el4")
                nc.vector.tensor_tensor(
                    out=zz[:], in0=psz[:], in1=albc[:], op=mybir.AluOpType.mult
                )
                nc.vector.reduce_sum(
                    out=el4[:], in_=zz[:].rearrange("p (h d) -> p h d", h=HEADS),
                    axis=mybir.AxisListType.X,
                )
                nc.vector.tensor_tensor(
                    out=zz[:], in0=psz[:], in1=arbc[:], op=mybir.AluOpType.mult
                )
                er4 = wp.tile([P, HEADS], F32, tag="er4")
                nc.vector.reduce_sum(
                    out=er4[:],
                    in_=zz[:].rearrange("p (h d) -> p h d", h=HEADS),
                    axis=mybir.AxisListType.X,
                )
                nc.vector.tensor_copy(
                    out=er_sb[:, t * HEADS : (t + 1) * HEADS], in_=er4[:]
                )
                zst = wp.tile([P, ZROW], DT, tag="zst")
                nc.vector.tensor_copy(out=zst[:, 0 : HEADS * HID], in_=psz[:])
                nc.vector.tensor_copy(
                    out=zst[:, HEADS * HID : HEADS * HID + HEADS], in_=el4[:]
                )
                nc.sync.dma_start(out=ag_z[t * P : t * P + w, :], in_=zst[:w])

            if _want("ag3"):
                nc.gpsimd.collective_compute(
                    "AllGather", mybir.AluOpType.bypass, replica_groups=rg,
                    ins=[ag_z[:].opt()], outs=[tab_z[:].opt()],
                )

            # ---------------- GAT edge phase ------------------------------
            for t in range(NTILES) if _want("gatedge") else []:
                w = TILE_W[t]
                ps_out = pp3.tile([P, HEADS * HID], F32, tag="mm", space="PSUM")
                ps_den = pp.tile([P, HEADS], F32, tag="den", space="PSUM")
                ert = er_sb[:, t * HEADS : (t + 1) * HEADS]
                for b in range(NB):
                    i0 = t * IDXW + b * GB * 8
                    zg = gp.tile([P, GB, ZROW], DT, tag="zg")
                    nc.gpsimd.dma_gather(
                        zg[:], tab_z[:], src16[:, i0 : i0 + GB * 8],
                        GB * P, GB * P, ZROW, elem_step=ZROW,
                        single_packet=False, queue_num=next_q(),
                    )
                    # transposed one-hots for the er expansion
                    g0 = (t * nchunks + b * GB) * P
                    dlT = gp.tile([P, GB * P], DT, tag="dlT")
                    nc.sync.dma_start(
                        out=dlT[:], in_=dstlocT_in[:, g0 : g0 + GB * P]
                    )
                    ohT = wp.tile([P, GB * P], DT, tag="ohT")
                    nc.vector.tensor_scalar(
                        out=ohT[:], in0=dlT[:], scalar1=iotac[:, 0:1],
                        scalar2=None, op0=mybir.AluOpType.is_equal,
                    )
                    ps_er = pp.tile([P, GB * HEADS], F32, tag="er", space="PSUM")
                    for cc in range(GB):
                        nc.tensor.matmul(
                            out=ps_er[:, cc * HEADS : (cc + 1) * HEADS],
                            lhsT=ohT[:, cc * P : (cc + 1) * P], rhs=ert,
                            start=True, stop=True, skip_group_check=True,
                        )
                    # batched ex = exp(lrelu(el + er))  [128, GB*4]
                    e_all = wp.tile([P, GB * HEADS], F32, tag="e_all")
                    ex_all = wp.tile([P, GB * HEADS], DT, tag="ex_all")
                    nc.vector.tensor_tensor(
                        out=e_all[:].rearrange("p (c h) -> p c h", h=HEADS),
                        in0=zg[:, :, HEADS * HID : HEADS * HID + HEADS],
                        in1=ps_er[:].rearrange("p (c h) -> p c h", h=HEADS),
                        op=mybir.AluOpType.add,
                    )
                    nc.vector.scalar_tensor_tensor(
                        out=e_all[:], in0=e_all[:], scalar=0.2,
                        in1=e_all[:], op0=mybir.AluOpType.mult,
                        op1=mybir.AluOpType.max,
                    )
                    nc.scalar.activation(
                        out=ex_all[:], in_=e_all[:],
                        func=mybir.ActivationFunctionType.Exp,
                    )
                    oh = onehot_batch(t, b)
                    for cc in range(GB):
                        cch = b * GB + cc
                        first = cch == 0
                        last = cch == nchunks - 1
                        zw = wp.tile([P, HEADS * HID], DT, tag="zw")
                        nc.vector.tensor_tensor(
                            out=zw[:].rearrange("p (h d) -> p h d", h=HEADS),
                            in0=zg[:, cc, 0 : HEADS * HID].rearrange(
                                "p (h d) -> p h d", h=HEADS
                            ),
                            in1=ex_all[:, cc * HEADS : (cc + 1) * HEADS]
                            .to_broadcast([P, HEADS, HID]),
                            op=mybir.AluOpType.mult,
                        )
                        nc.tensor.matmul(
                            out=ps_out[:], lhsT=oh[:, cc, :], rhs=zw[:],
                            start=first, stop=last,
                        )
                        nc.tensor.matmul(
                            out=ps_den[:], lhsT=oh[:, cc, :],
                            rhs=ex_all[:, cc * HEADS : (cc + 1) * HEADS],
                            start=first, stop=last,
                        )
                # epilogue: out = ps_out / den; x4 = mean_h + bg_mean
                den = wp.tile([P, HEADS], F32, tag="den_sb")
                nc.vector.tensor_scalar(
                    out=den[:], in0=ps_den[:], scalar1=1e-30, scalar2=None,
                    op0=mybir.AluOpType.max,
                )
                rden = wp.tile([P, HEADS], F32, tag="rden")
                nc.vector.reciprocal(out=rden[:], in_=den[:])
                outw = wp.tile([P, HEADS * HID], F32, tag="outw")
                nc.vector.tensor_tensor(
                    out=outw[:].rearrange("p (h d) -> p h d", h=HEADS),
                    in0=ps_out[:].rearrange("p (h d) -> p h d", h=HEADS),
                    in1=rden[:].to_broadcast([P, HEADS, HID]),
                    op=mybir.AluOpType.mult,
                )
                hsum = wp.tile([P, HID], F32, tag="hsum")
                nc.vector.tensor_tensor(
                    out=hsum[:], in0=outw[:, 0:HID], in1=outw[:, HID : 2 * HID],
                    op=mybir.AluOpType.add,
                )
                nc.vector.tensor_tensor(
                    out=hsum[:], in0=hsum[:], in1=outw[:, 2 * HID : 3 * HID],
                    op=mybir.AluOpType.add,
                )
                nc.vector.tensor_tensor(
                    out=hsum[:], in0=hsum[:], in1=outw[:, 3 * HID : 4 * HID],
                    op=mybir.AluOpType.add,
                )
                x4 = wp.tile([P, HID], F32, tag="x4")
                nc.vector.scalar_tensor_tensor(
                    out=x4[:], in0=hsum[:], scalar=1.0 / HEADS, in1=bgm[:],
                    op0=mybir.AluOpType.mult, op1=mybir.AluOpType.add,
                )
                nc.vector.tensor_scalar(
                    out=x4[:], in0=x4[:], scalar1=dsrc[:, t : t + 1],
                    scalar2=None, op0=mybir.AluOpType.mult,
                )
                m3 = wp.tile([P, HID], F32, tag="m3")
                nc.vector.tensor_tensor(
                    out=m3[:], in0=x4[:], in1=w3bc[:], op=mybir.AluOpType.mult
                )
                sv = wp.tile([P, 1], F32, tag="sv")
                nc.vector.reduce_sum(out=sv[:], in_=m3[:], axis=mybir.AxisListType.X)
                sst = wp.tile([P, SROW], DT, tag="sst")
                nc.vector.tensor_copy(out=sst[:, 0:1], in_=sv[:])
                nc.sync.dma_start(out=ag_s[t * P : t * P + w, :], in_=sst[:w])

            if _want("ag4"):
                nc.gpsimd.collective_compute(
                    "AllGather", mybir.AluOpType.bypass, replica_groups=rg,
                    ins=[ag_s[:].opt()], outs=[tab_s[:].opt()],
                )

            # ---------------- GC3 edge + sigmoid --------------------------
            for t in range(NTILES) if _want("gc3") else []:
                w = TILE_W[t]
                sg = gp.tile([P, nchunks, SROW], DT, tag="sg")
                for c0, c1 in _splits(nchunks, 2):
                    nc.gpsimd.dma_gather(
                        sg[:, c0:c1, :], tab_s[:],
                        src16[:, t * IDXW + c0 * 8 : t * IDXW + c1 * 8],
                        (c1 - c0) * P, (c1 - c0) * P, SROW, elem_step=SROW,
                        single_packet=False, queue_num=next_q(),
                    )
                ps = pp.tile([P, 1], F32, tag="den", space="PSUM")
                for b in range(NB):
                    oh = onehot_batch(t, b)
                    for cc in range(GB):
                        cch = b * GB + cc
                        nc.tensor.matmul(
                            out=ps[:], lhsT=oh[:, cc, :], rhs=sg[:, cch, 0:1],
                            start=(cch == 0), stop=(cch == nchunks - 1),
                        )
                rv = wp.tile([P, 1], F32, tag="rv")
                nc.vector.tensor_scalar(
                    out=rv[:], in0=ps[:], scalar1=ddst[:, t : t + 1],
                    scalar2=None, op0=mybir.AluOpType.mult,
                )
                nc.scalar.activation(
                    out=rv[:], in_=rv[:],
                    func=mybir.ActivationFunctionType.Sigmoid,
                    bias=b3bc[:, 0:1], scale=1.0,
                )
                nc.sync.dma_start(out=risk_out[t * P : t * P + w, :], in_=rv[:w])

            if stop_after != "all":
                bail()

    nc.compile()
    return nc


# ----------------------------------------------------------------------------
# host driver
# ----------------------------------------------------------------------------

def _get_program(nchunks):
    if nchunks not in _compiled_cache:
        _compiled_cache[nchunks] = _build(nchunks)
    return _compiled_cache[nchunks]


def _install_ntff_hook():
    """Profiling support: register the NTFF hook bass_utils expects when this
    image's antenv package lacks axon_hooks. Best-effort, trace-path only."""
    import types

    try:
        import antenv.axon_hooks  # noqa: F401

        return
    except ImportError:
        pass
    try:
        import antenv
        from trn_agent_boot.trn_boot import _ntff_profile_via_ctypes

        hook = _ntff_profile_via_ctypes("/opt/axon/libaxon_pjrt.so")
        mod = types.ModuleType("antenv.axon_hooks")
        mod.get_axon_ntff_profile_hook = lambda: hook
        mod.set_axon_ntff_profile_hook = lambda h: None
        sys.modules["antenv.axon_hooks"] = mod
        antenv.axon_hooks = mod
    except Exception:
        pass


def _to_table_dtype(a):
    if TABLE_BF16:
        import ml_dtypes

        return a.astype(ml_dtypes.bfloat16)
    return a.astype(np.float32)


def kernel(
    features, src, dst, W1, b1, W2, b2, W3, b3, Wg, attn_l, attn_r, bg,
    _trace=False,
):
    features = np.asarray(features, np.float32)
    per_core, deg_tiles, nchunks = _preprocess(src, dst)
    nc = _get_program(nchunks)

    iota4 = np.tile(np.arange(P, dtype=np.float32), nchunks)[None, :].repeat(P, 0)
    iotac = np.arange(P, dtype=np.float32)[:, None]
    common = dict(
        w1=np.asarray(W1, np.float32),
        w2=np.asarray(W2, np.float32),
        wg=np.asarray(Wg, np.float32),
        albc=np.tile(np.asarray(attn_l, np.float32).reshape(1, -1), (P, 1)),
        arbc=np.tile(np.asarray(attn_r, np.float32).reshape(1, -1), (P, 1)),
        b1bc=np.tile(np.asarray(b1, np.float32).reshape(1, -1), (P, 1)),
        b2bc=np.tile(np.asarray(b2, np.float32).reshape(1, -1), (P, 1)),
        bgbc=np.tile(np.asarray(bg, np.float32).reshape(1, -1), (P, 1)),
        w3bc=np.tile(np.asarray(W3, np.float32).reshape(1, -1), (P, 1)),
        b3bc=np.full((P, 1), np.float32(np.asarray(b3).reshape(-1)[0])),
        iota4=_to_table_dtype(iota4),
        iotac=iotac.astype(np.float32),
    )
    in_maps = []
    for c in range(NCORES):
        m = dict(common)
        m["xT"] = np.ascontiguousarray(features[c * NPC : (c + 1) * NPC].T)
        m["degout"], m["degin"] = deg_tiles[c]
        m["src16"] = per_core[c]["src16"]
        m["dstloc"] = _to_table_dtype(per_core[c]["dstloc"])
        # per-edge dstloc in wrapped-by-chunk row layout, replicated down parts
        dl = per_core[c]["dstloc"]  # [128, NTILES*nchunks], [p, g] = edge g*128+p
        row = dl.T.reshape(1, -1)   # [1, NTILES*nchunks*128] edge-major
        m["dstlocT"] = _to_table_dtype(np.repeat(row, P, axis=0))
        in_maps.append(m)

    if _trace:
        _install_ntff_hook()
    res = bass_utils.run_bass_kernel_spmd(
        nc, in_maps, core_ids=list(range(NCORES)), trace=_trace
    )
    out = np.concatenate([res.results[c]["risk"] for c in range(NCORES)], axis=0)
    if _trace:
        kernel.last_exec_time_ns = res.exec_time_ns
        kernel.last_results = res
    return out.astype(np.float32)
